# revision 1
# baseline (speedup 1.0000x reference)
"""Trainium2 Bass kernel for a BAN (bilinear attention network) layer.

Reference computation (per batch b, head h, hd=64, scale=hd**-0.5):
    vp = (v @ Wv + bv)  -> [V=1024, 512] split into heads [h, V, 64]
    qp = (q @ Wq + bq)  -> [Q=512, 512]  split into heads [h, Q, 64]
    logits = vp_h @ att_w_h @ qp_h^T * scale        [V, Q]
    w = softmax(logits, axis=-1)
    pooled_v = mean_v(w @ qp_h)          [64]
    pooled_q = mean_q(w^T @ vp_h)        [64]
    fused = concat per head [pooled_v, pooled_q] -> [1024]
    out = relu(fused @ Wo + bo)          [512]

Key algebraic simplifications used here (validated to 4e-7 rel err vs ref):
  * rows of w sum to 1 => pooled_q = (1/Q) * colsum_v(vp_h)
      (does not depend on the attention at all)
  * pooled_v = (colmean_v w) @ qp_h; with e = exp(logits), s = rowsum(e):
      z = (1/V) * sum_v e[v,:] / s[v]  (computed as a tiny TensorE matmul
      with the reciprocal rowsums as the stationary operand)
      pooled_v = z @ (q @ Wq)_h + bq_h
  * att_w and the 1/8 scale are folded into Wq on the host (weight-only
    transform): Wqw[d, h*64+i] = scale * sum_j Wq[d, h*64+j] att_w[h,i,j]

Sharding: data-parallel over batch, 2 batches per core, params replicated,
no collectives.  Host does only layout transforms (transposes / reshapes /
weight folding / bf16 cast); all O(activation*weight) math runs on device.
"""

import numpy as np
import ml_dtypes

BF16 = ml_dtypes.bfloat16

B, V_NUM, Q_NUM = 16, 1024, 512
V_DIM, Q_DIM = 256, 128
HIDDEN, HEADS, HD = 512, 8, 64
SCALE = HD ** -0.5

N_CORES = 8
BPC = B // N_CORES          # batches per core
DC = V_DIM // 128           # d-chunks of v (2)
IB = HIDDEN // 128          # i-blocks of hidden (4)
QC = Q_NUM // 128           # q-chunks (4)
VB = V_NUM // 512           # v-blocks of 512 (2)
VCH = V_NUM // 128          # v-chunks of 128 (8)
NB = HIDDEN // 128          # out feature blocks (4)
KC = (2 * HEADS * HD) // 128  # fused feature chunks of 128 (8)

_CACHE = {}


def _build_nc():
    from contextlib import ExitStack

    import concourse.bass as bass
    import concourse.tile as tile
    from concourse import bacc, mybir
    from concourse.masks import make_identity
    from concourse.tile import add_dep_helper

    f32 = mybir.dt.float32
    bf16 = mybir.dt.bfloat16
    fp8 = mybir.dt.float8e4
    AF = mybir.ActivationFunctionType
    ALU = mybir.AluOpType
    AX = mybir.AxisListType

    nc = bacc.Bacc("TRN2", target_bir_lowering=False)

    # ALL inputs ride in one [128, 11312] bf16 param -> a single input DMA
    # (one DMA fans out across all 16 SDMA engines, and every later
    # data-dependent DMA then gets a fresh HWDGE lane so its trigger only
    # carries its one data-dep wait; the kernel-tail drain also stays under
    # its wait-slot limit).  f32 biases ride as raw bits, bitcast on device.
    PCK_VT, PCK_QT = 0, BPC * DC * V_NUM
    PCK_WALL = PCK_QT + BPC * Q_NUM
    PCK_ID = PCK_WALL + (4 + KC) * HIDDEN
    PCK_BALL = PCK_ID + 8
    PCK_COLS = PCK_BALL + 2 * (2 * IB + HEADS + NB)
    packed_p = nc.declare_dram_parameter("packed", [128, PCK_COLS], bf16, isOutput=False)
    outT_p = nc.declare_dram_parameter("outT", [HIDDEN, BPC], f32, isOutput=True)

    with tile.TileContext(nc) as tc, ExitStack() as ctx:
        const = ctx.enter_context(tc.tile_pool(name="const", bufs=1))
        work = ctx.enter_context(tc.tile_pool(name="work", bufs=1))
        epool = ctx.enter_context(tc.tile_pool(name="epool", bufs=128))
        spool = ctx.enter_context(tc.tile_pool(name="spool", bufs=32))
        dpool = ctx.enter_context(tc.tile_pool(name="dpool", bufs=2, space="DRAM"))
        ps_big = ctx.enter_context(tc.tile_pool(name="ps_big", bufs=4, space="PSUM"))
        ps_z = ctx.enter_context(tc.tile_pool(name="ps_z", bufs=2, space="PSUM"))
        ps_tr = ctx.enter_context(tc.tile_pool(name="ps_tr", bufs=1, space="PSUM"))
        ps_sm = ctx.enter_context(tc.tile_pool(name="ps_sm", bufs=1, space="PSUM"))

        class SlotGuard:
            """Explicit WAR edges for psum slot reuse: the first writer of
            allocation i+bufs must wait for all readers of allocation i."""

            def __init__(self, bufs):
                self.bufs = bufs
                self.hist = []

            def alloc(self):
                self.hist.append([None, []])
                return len(self.hist) - 1

            def writer(self, idx, mi):
                if self.hist[idx][0] is None:
                    self.hist[idx][0] = mi
                    prev = idx - self.bufs
                    if prev >= 0:
                        for r in self.hist[prev][1]:
                            add_dep_helper(mi.ins, r.ins, sync=True,
                                           reason="psum slot WAR guard")
                return mi

            def reader(self, idx, mi):
                self.hist[idx][1].append(mi)
                return mi

        g_big = SlotGuard(4)
        g_z = SlotGuard(2)
        g_tr = SlotGuard(1)
        g_sm = SlotGuard(1)

        # ---- constants + inputs: one DMA, sliced views ----
        packed_sb = const.tile([128, PCK_COLS], bf16, tag="packed")
        nc.sync.dma_start(packed_sb[:], packed_p[:])
        vt_sb = packed_sb[:, PCK_VT:PCK_QT].rearrange(
            "p (b c v) -> p b c v", b=BPC, c=DC)
        qt_sb = packed_sb[:, PCK_QT:PCK_WALL].rearrange(
            "p (b q) -> p b q", b=BPC)
        wall_sb = packed_sb[:, PCK_WALL:PCK_ID].rearrange(
            "p (w h) -> p w h", w=4 + KC)
        wv_sb = wall_sb[:, 0:DC]
        wqw_sb = wall_sb[:, DC]
        wq_sb = wall_sb[:, DC + 1]
        wo_sb = wall_sb[:, DC + 2:DC + 2 + KC]
        ident_sb = packed_sb[0:8, PCK_ID:PCK_ID + 8]
        ball_sb = packed_sb[:, PCK_BALL:PCK_COLS].bitcast(f32)
        bv_sb = ball_sb[:, 0:IB]
        bqw_sb = ball_sb[:, IB:2 * IB]
        fb_sb = ball_sb[:, 2 * IB:2 * IB + HEADS]
        bo_sb = ball_sb[:, 2 * IB + HEADS:]

        # ---- long-lived activations ----
        vpT_sb = work.tile([128, BPC, IB, V_NUM], bf16, tag="vpt")
        qpwT_sb = work.tile([128, BPC, IB, Q_NUM], bf16, tag="qpwt")
        qp_sb = work.tile([128, BPC, QC, HIDDEN], bf16, tag="qp")
        zstack_sb = work.tile([8, BPC, Q_NUM], bf16, tag="zstack")
        zrow_sb = work.tile([1, BPC, HEADS, Q_NUM], bf16, tag="zrow")
        zT_sb = work.tile([128, BPC, QC, HEADS], bf16, tag="zT")
        fusedT_sb = work.tile([128, KC, BPC], bf16, tag="fused")
        outT_sb = work.tile([128, NB, BPC], f32, tag="outT")
        cv_sb = work.tile([128, BPC, DC], f32, tag="cv")
        cvb_sb = work.tile([128, BPC, DC], bf16, tag="cvb")


        # ---- prologue: projections. Returns a list of small thunks (one
        # per psum group) so the caller can spread them through another
        # batch's main loop, filling PE idle slots. ----
        def prologue_thunks(b):
            thunks = []

            def th(fn):
                thunks.append(fn)
            # per-ib groups first (pair t of the main loop only needs ib=t)
            for ib in range(IB):
                for vb in range(VB):
                    def vpt_group(ib=ib, vb=vb):
                        ps = ps_big.tile([128, 512], f32, tag="big")
                        gi = g_big.alloc()
                        for dc in range(DC):
                            g_big.writer(gi, nc.tensor.matmul(
                                ps[:], lhsT=wv_sb[:, dc, ib * 128:(ib + 1) * 128],
                                rhs=vt_sb[:, b, dc, vb * 512:(vb + 1) * 512],
                                start=(dc == 0), stop=(dc == DC - 1)))
                        g_big.reader(gi, nc.vector.tensor_scalar_add(
                            vpT_sb[:, b, ib, vb * 512:(vb + 1) * 512], ps[:],
                            bv_sb[:, ib:ib + 1]))
                    th(vpt_group)

                def qpwt_group(ib=ib):
                    ps = ps_big.tile([128, 512], f32, tag="big")
                    gi = g_big.alloc()
                    g_big.writer(gi, nc.tensor.matmul(
                        ps[:], lhsT=wqw_sb[:, ib * 128:(ib + 1) * 128],
                        rhs=qt_sb[:, b, :], start=True, stop=True))
                    g_big.reader(gi, nc.vector.tensor_scalar_add(
                        qpwT_sb[:, b, ib, :], ps[:], bqw_sb[:, ib:ib + 1]))
                th(qpwt_group)
            # the rest feeds only the z tail / epilogue
            for qc in range(QC):
                def qp_group(qc=qc):
                    ps = ps_big.tile([128, 512], f32, tag="big")
                    gi = g_big.alloc()
                    g_big.writer(gi, nc.tensor.matmul(
                        ps[:], lhsT=qt_sb[:, b, qc * 128:(qc + 1) * 128],
                        rhs=wq_sb[:], start=True, stop=True))
                    g_big.reader(gi, nc.vector.tensor_copy(
                        qp_sb[:, b, qc, :], ps[:]))
                th(qp_group)

            def cv_group():
                for dc in range(DC):
                    nc.vector.tensor_reduce(
                        cv_sb[:, b, dc:dc + 1], vt_sb[:, b, dc, :],
                        axis=AX.X, op=ALU.add)
                    nc.vector.tensor_copy(
                        cvb_sb[:, b, dc:dc + 1], cv_sb[:, b, dc:dc + 1])
            th(cv_group)
            for ib in range(IB):
                for half in range(2):
                    def pq_group(ib=ib, half=half):
                        h = 2 * ib + half
                        psq = ps_sm.tile([128, 8], f32, tag="sm")
                        gi = g_sm.alloc()
                        for dc in range(DC):
                            g_sm.writer(gi, nc.tensor.matmul(
                                psq[64:128, 0:1],
                                lhsT=wv_sb[:, dc, ib * 128 + 64 * half: ib * 128 + 64 * half + 64],
                                rhs=cvb_sb[:, b, dc:dc + 1],
                                start=(dc == 0), stop=(dc == DC - 1)))
                        g_sm.reader(gi, nc.vector.tensor_scalar(
                            fusedT_sb[64:128, h, b:b+1], psq[64:128, 0:1],
                            1.0 / Q_NUM, fb_sb[64:128, h:h + 1],
                            ALU.mult, ALU.add))
                    th(pq_group)
            return thunks

        # ---- main loop ----
        # Software pipeline: pair t's logits/exp matmuls are interleaved with
        # pair t-1's colsum matmuls so the in-order PE always has fill work
        # while waiting for ScalarE to drain logits psum slots (also keeps
        # the PE HAM-warm).  Each batch's z tail (restack/transpose/pooled_v)
        # is emitted inside the NEXT batch's stream to overlap with its exps.
        def emit_colsum(b, side):
            h, hb, s_t, e_list, rb_t = side
            zps = ps_z.tile([1, 512], f32, tag="z")
            gi = g_z.alloc()
            mms = []
            for c in range(VCH):
                mms.append((gi, zps, c))
            return (h, zps, gi, e_list, rb_t, mms)

        def ztail_thunks(b, zrow_insts):
            thunks = []

            def th(fn):
                thunks.append(fn)

            def restack():
                # restack z rows from partition 0 onto partitions 0..7
                # (SBUF->SBUF DMA; both APs stream in the same h-major order)
                nc.sync.dma_start(
                    zstack_sb[:, b, :],
                    zrow_sb[0:1, b].rearrange("p h q -> p (h q)"))
                for qc in range(QC):
                    pst = ps_tr.tile([128, 8], bf16, tag="tr")
                    gi = g_tr.alloc()
                    g_tr.writer(gi, nc.tensor.transpose(
                        pst[:], zstack_sb[:, b, qc * 128:(qc + 1) * 128],
                        ident_sb[:]))
                    g_tr.reader(gi, nc.vector.tensor_copy(
                        zT_sb[:, b, qc, :], pst[:]))
            th(restack)
            for h in range(HEADS):
                def pv_group(h=h):
                    psv = ps_sm.tile([128, 8], f32, tag="sm")
                    gi = g_sm.alloc()
                    for qc in range(QC):
                        g_sm.writer(gi, nc.tensor.matmul(
                            psv[0:64, 0:1],
                            lhsT=qp_sb[:, b, qc, h * 64:(h + 1) * 64],
                            rhs=zT_sb[:, b, qc, h:h + 1],
                            start=(qc == 0), stop=(qc == QC - 1)))
                    g_sm.reader(gi, nc.vector.tensor_scalar(
                        fusedT_sb[0:64, h, b:b+1], psv[0:64, 0:1],
                        float(2.0 ** -18), fb_sb[0:64, h:h + 1],
                        ALU.mult, ALU.add))
                th(pv_group)
            return thunks

        def emit_main(b, pre_work):
            """pre_work: list of thunks to emit early in this batch's stream
            (previous batch's z tail) so they overlap with this batch's exps."""
            zrow_insts = []
            pending = None

            def drain_pending_tail(pend):
                for h, zps, gi, e_list, rb_t in pend:
                    zri = g_z.reader(gi, nc.vector.tensor_copy(
                        zrow_sb[0:1, b, h, :], zps[:]))
                    zrow_insts.append(zri)

            for t in range(HEADS // 2):
                hA, hB = 2 * t, 2 * t + 1
                sides = []
                for h in (hA, hB):
                    s_t = spool.tile([128, VCH], f32, tag="s")
                    sides.append([h, 64 * (h % 2), s_t, []])
                # pending colsum state
                pend_state = None
                if pending is not None:
                    pend_state = []
                    for h, hb, s_t, e_list, rb_t in pending:
                        zps = ps_z.tile([1, 512], f32, tag="z")
                        gi = g_z.alloc()
                        pend_state.append((h, zps, gi, e_list, rb_t))
                for c in range(VCH):
                    for side in sides:
                        h, hb, s_t, e_list = side
                        ps = ps_big.tile([128, 512], f32, tag="big")
                        gi = g_big.alloc()
                        g_big.writer(gi, nc.tensor.matmul(
                            ps[:],
                            lhsT=vpT_sb[hb:hb + 64, b, t, c * 128:(c + 1) * 128],
                            rhs=qpwT_sb[hb:hb + 64, b, t, :],
                            start=True, stop=True))
                        e_t = epool.tile([128, 512], fp8, tag="e")
                        g_big.reader(gi, nc.scalar.activation(
                            e_t[:], ps[:], AF.Exp, accum_out=s_t[:, c:c + 1]))
                        e_list.append(e_t)
                    if pend_state is not None:
                        for h, zps, gi, e_list, rb_t in pend_state:
                            g_z.writer(gi, nc.tensor.matmul(
                                zps[:], lhsT=rb_t[:, c:c + 1], rhs=e_list[c][:],
                                start=(c == 0), stop=(c == VCH - 1)))
                    for _ in range(2):
                        if pre_work:
                            pre_work.pop(0)()
                if pend_state is not None:
                    drain_pending_tail(pend_state)
                # reciprocals for this pair (DVE, overlaps next pair)
                new_pending = []
                for h, hb, s_t, e_list in sides:
                    r_t = spool.tile([128, VCH], f32, tag="r")
                    nc.vector.reciprocal(r_t[:], s_t[:])
                    rb_t = spool.tile([128, VCH], fp8, tag="rb")
                    nc.vector.tensor_scalar_mul(
                        rb_t[:], r_t[:], float(2 ** 18) / V_NUM)
                    new_pending.append((h, hb, s_t, e_list, rb_t))
                pending = new_pending
            # last pair's colsum (not interleaved)
            pend_state = []
            for h, hb, s_t, e_list, rb_t in pending:
                zps = ps_z.tile([1, 512], f32, tag="z")
                gi = g_z.alloc()
                pend_state.append((h, zps, gi, e_list, rb_t))
            for c in range(VCH):
                for h, zps, gi, e_list, rb_t in pend_state:
                    g_z.writer(gi, nc.tensor.matmul(
                        zps[:], lhsT=rb_t[:, c:c + 1], rhs=e_list[c][:],
                        start=(c == 0), stop=(c == VCH - 1)))
            drain_pending_tail(pend_state)
            while pre_work:
                pre_work.pop(0)()
            return zrow_insts

        pro0 = prologue_thunks(0)
        for fn in pro0[:3]:
            fn()
        zrows0 = emit_main(0, pro0[3:] + prologue_thunks(1))
        zrows1 = emit_main(1, ztail_thunks(0, zrows0))
        for fn in ztail_thunks(1, zrows1):
            fn()

        # ---- epilogue: out = relu(fused @ Wo + bo), computed transposed ----
        for nb in range(NB):
            pso = ps_sm.tile([128, 8], f32, tag="sm")
            gi = g_sm.alloc()
            for kc in range(KC):
                g_sm.writer(gi, nc.tensor.matmul(
                    pso[:, 0:BPC],
                    lhsT=wo_sb[:, kc, nb * 128:(nb + 1) * 128],
                    rhs=fusedT_sb[:, kc, :],
                    start=(kc == 0), stop=(kc == KC - 1)))
            g_sm.reader(gi, nc.scalar.activation(
                outT_sb[:, nb, :], pso[:, 0:BPC], AF.Relu,
                bias=bo_sb[:, nb:nb + 1]))
        nc.sync.dma_start(
            outT_p[:].rearrange("(o p) b -> p o b", p=128), outT_sb[:])

    nc.compile()
    return nc


def _get_nc():
    if "nc" not in _CACHE:
        _CACHE["nc"] = _build_nc()
    return _CACHE["nc"]


def _host_prep(v, q, Wv, bv, Wq, bq, att_w, Wo, bo):
    """Host-side layout transforms + weight folding. Returns per-core in_maps."""
    v = np.asarray(v, np.float32)
    q = np.asarray(q, np.float32)
    Wv = np.asarray(Wv, np.float32)
    bv = np.asarray(bv, np.float32)
    Wq = np.asarray(Wq, np.float32)
    bq = np.asarray(bq, np.float32)
    att_w = np.asarray(att_w, np.float32)
    Wo = np.asarray(Wo, np.float32)
    bo = np.asarray(bo, np.float32)

    # fold att_w and softmax scale into the q projection
    Wq_h = Wq.reshape(Q_DIM, HEADS, HD)
    Wqw = (SCALE * np.einsum("dhj,hij->dhi", Wq_h, att_w)).reshape(Q_DIM, HIDDEN)
    bqw = (SCALE * np.einsum("hj,hij->hi", bq.reshape(HEADS, HD), att_w)).reshape(HIDDEN)

    wall = np.concatenate([
        Wv.reshape(DC, 128, HIDDEN).transpose(1, 0, 2),
        Wqw.reshape(1, 128, HIDDEN).transpose(1, 0, 2),
        Wq.reshape(1, 128, HIDDEN).transpose(1, 0, 2),
        Wo.reshape(KC, 128, HIDDEN).transpose(1, 0, 2),
    ], axis=1).reshape(128, (4 + KC) * HIDDEN)
    fbias = np.concatenate(
        [bq.reshape(HEADS, HD).T,
         (V_NUM / Q_NUM) * bv.reshape(HEADS, HD).T], axis=0)
    ball = np.concatenate([
        bv.reshape(IB, 128).T, bqw.reshape(IB, 128).T,
        fbias, bo.reshape(NB, 128).T], axis=1).astype(np.float32)
    ident = np.zeros((128, 8), np.float32)
    ident[:8, :8] = np.eye(8)
    shared_cols = np.concatenate([
        wall.astype(BF16), ident.astype(BF16),
        np.ascontiguousarray(ball).view(BF16)], axis=1)
    in_maps = []
    for i in range(N_CORES):
        sl = slice(i * BPC, (i + 1) * BPC)
        vt = v[sl].transpose(0, 2, 1).reshape(BPC, DC, 128, V_NUM)
        vt = vt.transpose(2, 0, 1, 3).reshape(128, BPC * DC * V_NUM)
        qt = q[sl].transpose(0, 2, 1).transpose(1, 0, 2).reshape(128, BPC * Q_NUM)
        packed = np.concatenate(
            [vt.astype(BF16), qt.astype(BF16), shared_cols], axis=1)
        in_maps.append({"packed": np.ascontiguousarray(packed)})
    return in_maps


def kernel(**inputs):
    from concourse.bass_utils import run_bass_kernel_spmd

    nc = _get_nc()
    in_maps = _host_prep(**inputs)
    res = run_bass_kernel_spmd(nc, in_maps, core_ids=list(range(N_CORES)))
    out = np.empty((B, HIDDEN), np.float32)
    for i in range(N_CORES):
        out[i * BPC:(i + 1) * BPC] = np.asarray(res.results[i]["outT"]).T
    return out

